# revision 1
# baseline (speedup 1.0000x reference)
"""Causal self-attention Bass kernel for Trainium2, 8-core data-parallel.

Problem: nn_CausalSelfAttention (B=8, T=1024, C=768, H=12, HD=64).
Sharding: pure data parallel over batch — each of the 8 NeuronCores
computes one batch element end-to-end; no collectives.

Per-core pipeline (matmuls in fp32r — full-rate rounded-fp32):
  1. PE-transpose x [T,C] -> xT [C,T]
  2. qT = Wq^T xT, kT = Wk^T xT  (feature-major, head-pair tiled)
     v  = x Wv  (token-major, packed per-head with a ones column -> v')
  3. Per head pair: sT = kT^T qT (scores transposed; both heads run
     concurrently on the PE via tile_position row-tiling), exp on the
     scalar engine (scale=1/8, no max subtraction — scores are O(1)),
     causal masking via persistent zero regions + a batched triangular
     mask multiply on GPSIMD, y'T = v'^T att^T accumulated over key
     chunks (the ones column of v' yields softmax denominators for
     free), then normalize via fast reciprocal + DMA broadcast.
  4. out = yT^T W_proj + b_proj.
"""

import numpy as np

B, T, C, H, HD = 8, 1024, 768, 12, 64
KT = C // 128   # 6 contraction tiles
TC = T // 128   # 8 token chunks
NPAIR = H // 2  # 6 head pairs


def build_nc(num_devices=8, reps=1):
    import concourse.bass as bass
    from concourse import bacc
    import concourse.mybir as mybir
    from concourse.tile import TileContext
    from concourse.masks import make_identity

    f32 = mybir.dt.float32
    f32r = mybir.dt.float32r
    Act = mybir.ActivationFunctionType
    AP = bass.AP

    nc = bacc.Bacc("TRN2", target_bir_lowering=False, debug=False,
                   num_devices=num_devices)

    x_d = nc.dram_tensor("x", [T, C], f32, kind="ExternalInput")
    wa_d = nc.dram_tensor("w_attn", [C, 3 * C], f32r, kind="ExternalInput")
    wp_d = nc.dram_tensor("w_proj", [C, C], f32r, kind="ExternalInput")
    ba_d = nc.dram_tensor("b_attn", [3 * C], f32, kind="ExternalInput")
    bp_d = nc.dram_tensor("b_proj", [C], f32, kind="ExternalInput")
    tri_d = nc.dram_tensor("tri", [128, 128], f32r, kind="ExternalInput")
    y_d = nc.dram_tensor("y", [T, C], f32, kind="ExternalOutput")

    with TileContext(nc) as tc:
        with (
            tc.tile_pool(name="const", bufs=1) as const,
            tc.tile_pool(name="persist", bufs=1) as persist,
        ):
            # ---------------- constants / prologue ----------------
            ident = const.tile([128, 128], f32)
            make_identity(nc, ident)
            ones1 = const.tile([1, 64], f32r)
            nc.vector.memset(ones1.bitcast(f32), 1.0)
            tri = const.tile([128, 128], f32r)
            nc.sync.dma_start(out=tri, in_=tri_d[:, :])
            # per-partition biases for q/k feature tiles: column a = head pair a
            ba_htd = ba_d[:].rearrange("(h t d) -> h t d", h=H, t=3)
            bq = const.tile([128, NPAIR], f32)
            bk = const.tile([128, NPAIR], f32)
            for a in range(NPAIR):
                nc.sync.dma_start(out=bq[:, a:a + 1],
                                  in_=ba_htd[2 * a:2 * a + 2, 0, :])
                nc.sync.dma_start(out=bk[:, a:a + 1],
                                  in_=ba_htd[2 * a:2 * a + 2, 1, :])
            # broadcast biases along free axis (v features / proj features)
            bv_b = const.tile([128, C], f32)
            ba_ap = ba_d[:]
            nc.sync.dma_start(
                out=bv_b[:, :].rearrange("p (h d) -> p h d", h=H),
                in_=AP(tensor=ba_ap.tensor, offset=ba_ap.offset + 2 * HD,
                       ap=[[0, 128], [3 * HD, H], [1, HD]]))
            bp_b = const.tile([128, C], f32)
            bp_ap = bp_d[:]
            nc.sync.dma_start(out=bp_b, in_=AP(tensor=bp_ap.tensor,
                                               offset=bp_ap.offset,
                                               ap=[[0, 128], [1, C]]))

            # ---------------- persistent big tensors ----------------
            qT = persist.tile([128, KT, T], f32r)       # q^T  head-pair-major
            kTt = persist.tile([128, KT, T], f32r)      # k^T
            vp = persist.tile([128, TC, H * 65], f32r)  # v' + ones column
            yT = persist.tile([128, KT, T], f32r)       # y^T

            nc.vector.memset(vp.bitcast(f32), 1.0)  # ones cols survive

            for _rep in range(reps):
                # ---------------- phase 1: xT + qkv (weights freed after) ----
                p1pool = tc.tile_pool(name="p1", bufs=1)
                p1 = p1pool.__enter__()
                xT = p1.tile([128, KT, T], f32r)            # x^T  [C, T]
                wq = p1.tile([128, KT, C], f32r)
                wk = p1.tile([128, KT, C], f32r)
                wv = p1.tile([128, KT, C], f32r)
                # W_attn loaded contiguously, head-major gather done on GPSIMD
                with tc.tile_pool(name="wnat", bufs=3) as wnat_p:
                    for kk in range(KT):
                        for half in range(2):
                            wn = wnat_p.tile([128, 6 * 192], f32r, name="wn")
                            nc.sync.dma_start(
                                out=wn,
                                in_=wa_d[128 * kk:128 * (kk + 1),
                                         1152 * half:1152 * (half + 1)])
                            for t_idx, w_t in ((0, wq), (1, wk), (2, wv)):
                                dst = w_t[:, kk, 384 * half:384 * (half + 1)] \
                                    .rearrange("p (h d) -> p h d", d=HD)
                                srcv = AP(tensor=wn.tensor,
                                          offset=wn.offset + HD * t_idx,
                                          ap=[wn.ap[0], [192, 6], [1, HD]])
                                nc.gpsimd.tensor_copy(dst, srcv)

                # ---------------- phase 1a: transpose x ----------------
                with (
                    tc.tile_pool(name="xnat", bufs=3) as xnat_p,
                    tc.tile_pool(name="tp", bufs=4, space="PSUM") as tp_p,
                ):
                    for i in range(TC):
                        xnat = xnat_p.tile([128, C], f32)
                        nc.sync.dma_start(out=xnat, in_=x_d[128 * i:128 * (i + 1), :])
                        for g in range(2):
                            tp = tp_p.tile([128, 384], f32)
                            for cc in range(3):
                                cb = 3 * g + cc
                                nc.tensor.transpose(
                                    tp[:, 128 * cc:128 * (cc + 1)],
                                    xnat[:, 128 * cb:128 * (cb + 1)], ident)
                            dst = AP(tensor=xT.tensor,
                                     offset=xT.offset + (3 * g) * T + 128 * i,
                                     ap=[xT.ap[0], [T, 3], [1, 128]])
                            src = AP(tensor=tp.tensor, offset=tp.offset,
                                     ap=[tp.ap[0], [128, 3], [1, 128]])
                            nc.vector.tensor_copy(dst, src)

                # ---------------- phase 1b: q^T, k^T, v' ----------------
                with tc.tile_pool(name="qkp", bufs=1, space="PSUM") as qkp:
                    for (w_t, dstT, bias, eng) in ((wq, qT, bq, "act"),
                                                   (wk, kTt, bk, "dve")):
                        for a in range(NPAIR):
                            ps0 = qkp.tile([128, 512], f32, tag="qk", name="ps0", bufs=4)
                            ps1 = qkp.tile([128, 512], f32, tag="qk", name="ps1", bufs=4)
                            for kk in range(KT):
                                st, sp = kk == 0, kk == KT - 1
                                lhsT = w_t[:, kk, 128 * a:128 * (a + 1)]
                                nc.tensor.matmul(ps0, lhsT, xT[:, kk, 0:512],
                                                 start=st, stop=sp)
                                nc.tensor.matmul(ps1, lhsT, xT[:, kk, 512:1024],
                                                 start=st, stop=sp)
                            for j, psj in ((0, ps0), (1, ps1)):
                                dst = dstT[:, a, 512 * j:512 * (j + 1)]
                                if eng == "act":
                                    nc.scalar.activation(dst, psj, Act.Identity,
                                                         bias=bias[:, a:a + 1])
                                else:
                                    nc.vector.tensor_scalar_add(dst, psj,
                                                                bias[:, a:a + 1])

                    # v token-major: lhsT = xT chunk, rhs = Wv
                    for i in range(TC):
                        pv = qkp.tile([128, C], f32, tag="v", name="pv", bufs=2)
                        for kk in range(KT):
                            st, sp = kk == 0, kk == KT - 1
                            lhsT = xT[:, kk, 128 * i:128 * (i + 1)]
                            nc.tensor.matmul(pv[:, 0:512], lhsT, wv[:, kk, 0:512],
                                             start=st, stop=sp)
                            nc.tensor.matmul(pv[:, 512:768], lhsT, wv[:, kk, 512:768],
                                             start=st, stop=sp)
                        dst = AP(tensor=vp.tensor, offset=vp.offset + i * (H * 65),
                                 ap=[vp.ap[0], [65, H], [1, HD]])
                        src = AP(tensor=pv.tensor, offset=pv.offset,
                                 ap=[pv.ap[0], [HD, H], [1, HD]])
                        bsrc = AP(tensor=bv_b.tensor, offset=bv_b.offset,
                                  ap=[bv_b.ap[0], [HD, H], [1, HD]])
                        nc.vector.tensor_add(dst, src, bsrc)

                p1pool.__exit__(None, None, None)

                # ---------------- phase 2: attention ----------------
                p2pool = tc.tile_pool(name="p2", bufs=1)
                p2 = p2pool.__enter__()
                wp = p2.tile([128, KT, C], f32r)
                for kk in range(KT):
                    nc.sync.dma_start(out=wp[:, kk, :],
                                      in_=wp_d[128 * kk:128 * (kk + 1), :])
                # attention probability buffers; [s] = head parity slot.
                # Zero regions above the causal diagonal stay zero forever.
                es0 = [p2.tile([128, 4 * 512], f32r, name=f"es0_{s}")
                       for s in range(2)]
                es1 = [p2.tile([128, 8 * 512], f32r, name=f"es1_{s}")
                       for s in range(2)]
                for s in range(2):
                    nc.vector.memset(es0[s].bitcast(f32), 0.0)
                    nc.vector.memset(es1[s].bitcast(f32), 0.0)
                with (
                    tc.tile_pool(name="sp", bufs=3, space="PSUM") as sp_p,
                    tc.tile_pool(name="yp", bufs=2, space="PSUM") as yp_p,
                    tc.tile_pool(name="rec", bufs=4) as rec_p,
                    tc.tile_pool(name="ysb", bufs=6) as ysb_p,
                ):
                    for a in range(NPAIR):
                        for j in range(2):
                            nm = 4 * (j + 1)          # causal k-chunks for q tile j
                            es = es0 if j == 0 else es1
                            for w0 in range(0, nm, 2):   # waves of 2 k-chunks
                                spt = [sp_p.tile([128, 1024], f32, tag="s",
                                                 name=f"spt{s}") for s in range(2)]
                                for s in range(2):
                                    r0, r1 = 64 * s, 64 * (s + 1)
                                    for mi, m in enumerate((w0, w0 + 1)):
                                        nc.tensor.matmul(
                                            spt[s][:, 512 * mi:512 * (mi + 1)],
                                            kTt[r0:r1, a, 128 * m:128 * (m + 1)],
                                            qT[r0:r1, a, 512 * j:512 * (j + 1)],
                                            start=True, stop=True,
                                            tile_position=(64 * s, 0))
                                for s in range(2):
                                    nc.scalar.activation(
                                        es[s][:, 512 * w0:512 * (w0 + 2)], spt[s],
                                        Act.Exp, scale=0.125)
                                # causal fixups for this wave (zero prefixes + tri)
                                for s in range(2):
                                    est = es[s]
                                    for m in (w0, w0 + 1):
                                        qs = 128 * m - 512 * j
                                        if qs > 0:
                                            nc.gpsimd.memset(
                                                est[:, 512 * m:512 * m + qs]
                                                .bitcast(f32), 0.0)
                                    if w0 >= 4 * j:
                                        base = 640 * w0 - (512 if j else 0)
                                        dview = AP(tensor=est.tensor,
                                                   offset=est.offset + base,
                                                   ap=[est.ap[0], [640, 2], [1, 128]])
                                        tview = AP(tensor=tri.tensor,
                                                   offset=tri.offset,
                                                   ap=[tri.ap[0], [0, 2], [1, 128]])
                                        nc.gpsimd.tensor_mul(dview, dview, tview)

                            # y'T = v'^T att^T accumulated over k-chunks
                            for s in range(2):
                                hh = 2 * a + s
                                est = es[s]
                                ypsum = yp_p.tile([128, 512], f32, tag="y",
                                                  name=f"ypsum{s}")
                                for m in range(nm):
                                    qs = min(max(128 * m - 512 * j, 0), 256)
                                    nc.tensor.matmul(
                                        ypsum[0:65, qs:512],
                                        vp[:, m, 65 * hh:65 * hh + 65],
                                        est[:, 512 * m + qs:512 * (m + 1)],
                                        start=(m == 0), stop=(m == nm - 1))
                                # normalize this (head, q-tile) half
                                # copy y' out of PSUM immediately (frees the bank),
                                # normalize later from SBUF
                                ysb = ysb_p.tile([65, 512], f32, name="ysb")
                                nc.vector.tensor_copy(ysb, ypsum[0:65, :])
                                rec = rec_p.tile([1, 512], f32r, name="rec")
                                with nc.allow_low_precision("softmax denom recip in f32r"):
                                    nc.vector.reciprocal(rec, ysb[64:65, :])
                                bc = yp_p.tile([128, 512], f32, tag="y", name="bc")
                                nc.tensor.matmul(bc[0:64, :], ones1, rec,
                                                 start=True, stop=True)
                                nc.vector.tensor_mul(
                                    yT[64 * s:64 * (s + 1), a, 512 * j:512 * (j + 1)],
                                    ysb[0:64, :], bc[0:64, :])

                # ---------------- phase 3: output projection ----------------
                with (
                    tc.tile_pool(name="pp", bufs=3, space="PSUM") as pp_p,
                    tc.tile_pool(name="osb", bufs=3) as osb_p,
                ):
                    for i in range(TC):
                        po = pp_p.tile([128, C], f32)
                        for kk in range(KT):
                            st, sp = kk == 0, kk == KT - 1
                            lhsT = yT[:, kk, 128 * i:128 * (i + 1)]
                            nc.tensor.matmul(po[:, 0:512], lhsT, wp[:, kk, 0:512],
                                             start=st, stop=sp)
                            nc.tensor.matmul(po[:, 512:768], lhsT, wp[:, kk, 512:768],
                                             start=st, stop=sp)
                        osb = osb_p.tile([128, C], f32)
                        nc.vector.tensor_add(osb, po, bp_b)
                        nc.sync.dma_start(out=y_d[128 * i:128 * (i + 1), :], in_=osb)
                p2pool.__exit__(None, None, None)

    nc.compile()
    return nc


_NC_CACHE = {}


def _get_nc():
    if "nc" not in _NC_CACHE:
        _NC_CACHE["nc"] = build_nc()
    return _NC_CACHE["nc"]


def kernel(x, W_attn, b_attn, W_proj, b_proj):
    from concourse.bass_utils import run_bass_kernel_spmd

    nc = _get_nc()
    x = np.asarray(x, dtype=np.float32)
    tri = np.triu(np.ones((128, 128), dtype=np.float32))
    shared = {
        "w_attn": np.ascontiguousarray(np.asarray(W_attn, dtype=np.float32)),
        "w_proj": np.ascontiguousarray(np.asarray(W_proj, dtype=np.float32)),
        "b_attn": np.ascontiguousarray(np.asarray(b_attn, dtype=np.float32)),
        "b_proj": np.ascontiguousarray(np.asarray(b_proj, dtype=np.float32)),
        "tri": tri,
    }
    in_maps = [dict(shared, x=np.ascontiguousarray(x[c])) for c in range(B)]
    res = run_bass_kernel_spmd(nc, in_maps, list(range(B)))
    out = np.stack([res.results[c]["y"] for c in range(B)], axis=0)
    return out.astype(np.float32)



# revision 11
# speedup vs baseline: 1.4155x; 1.4155x over previous
"""Causal self-attention Bass kernel for Trainium2, 8-core data-parallel.

Problem: nn_CausalSelfAttention (B=8, T=1024, C=768, H=12, HD=64).
Sharding: pure data parallel over batch — each of the 8 NeuronCores
computes one batch element end-to-end; no collectives.

Per-core pipeline (matmuls in fp32r — full-rate rounded-fp32):
  1. PE-transpose x [T,C] -> xT [C,T]
  2. qT/kT feature-major from the NATURAL W_attn layout: each head's
     q|k 128-col block is contiguous at col 192*h, so a pair of
     column-tiled M=64 matmuls (tile_position (0,0)/(0,64)) builds the
     head-pair-major psum directly — no weight gather pass.
     v token-major with a strided rhs AP over the natural tile,
     packed per-head with a ones column -> v'.
  3. Per head pair (probability buffers double-buffered across pairs):
     sT = kT^T qT via tile_position row-tiling, exp on the scalar
     engine, causal masking via per-wave zero prefixes (GPSIMD) + a
     per-block triangular mask multiply (DVE), y'T = v'^T att^T
     accumulated over key chunks (the ones column of v' yields softmax
     denominators for free). y' lands UNNORMALIZED in yT; denominator
     rows are DMA'd into a partition-packed tile, inverted with one
     cheap reciprocal_approx_fast per pair, DMA-broadcast across 64
     partitions, and multiplied in at the end.
  4. out = yT^T W_proj + b_proj.
"""

import numpy as np

B, T, C, H, HD = 8, 1024, 768, 12, 64
KT = C // 128   # 6 contraction tiles
TC = T // 128   # 8 token chunks
NPAIR = H // 2  # 6 head pairs


def build_nc(num_devices=8, reps=1):
    import concourse.bass as bass
    from concourse import bacc
    import concourse.mybir as mybir
    from concourse.tile import TileContext
    from concourse.masks import make_identity

    f32 = mybir.dt.float32
    f32r = mybir.dt.float32r
    bf16 = mybir.dt.bfloat16
    Act = mybir.ActivationFunctionType
    AP = bass.AP

    nc = bacc.Bacc("TRN2", target_bir_lowering=False, debug=False,
                   num_devices=num_devices)

    x_d = nc.dram_tensor("x", [T, C], f32, kind="ExternalInput")
    wa_d = nc.dram_tensor("w_attn", [C, 3 * C], f32r, kind="ExternalInput")
    wp_d = nc.dram_tensor("w_proj", [C, C], f32r, kind="ExternalInput")
    ba_d = nc.dram_tensor("b_attn", [3 * C], f32, kind="ExternalInput")
    bp_d = nc.dram_tensor("b_proj", [C], f32, kind="ExternalInput")
    tri_d = nc.dram_tensor("tri", [128, 128], f32r, kind="ExternalInput")
    y_d = nc.dram_tensor("y", [T, C], f32, kind="ExternalOutput")

    with TileContext(nc) as tc:
        with (
            tc.tile_pool(name="const", bufs=1) as const,
            tc.tile_pool(name="persist", bufs=1) as persist,
        ):
            # ---------------- constants / prologue ----------------
            ident = const.tile([128, 128], f32)
            make_identity(nc, ident)
            tri = const.tile([128, 128], f32r)
            nc.sync.dma_start(out=tri, in_=tri_d[:, :])
            tri_bf = const.tile([128, 128], bf16)
            nc.vector.tensor_copy(tri_bf, tri.bitcast(f32))
            # per-partition q|k biases in natural psum layout: column h =
            # [q bias of head h (rows 0-63), k bias (rows 64-127)]
            ba_ap = ba_d[:]
            bqk = const.tile([128, H], f32)
            for h in range(H):
                nc.sync.dma_start(
                    out=bqk[:, h:h + 1],
                    in_=AP(tensor=ba_ap.tensor, offset=ba_ap.offset + 192 * h,
                           ap=[[1, 128], [0, 1]]))
            # broadcast biases along free axis (v features / proj features)
            bv_b = const.tile([128, C], f32)
            nc.sync.dma_start(
                out=bv_b[:, :].rearrange("p (h d) -> p h d", h=H),
                in_=AP(tensor=ba_ap.tensor, offset=ba_ap.offset + 2 * HD,
                       ap=[[0, 128], [3 * HD, H], [1, HD]]))
            bp_b = const.tile([128, C], f32)
            bp_ap = bp_d[:]
            nc.sync.dma_start(out=bp_b, in_=AP(tensor=bp_ap.tensor,
                                               offset=bp_ap.offset,
                                               ap=[[0, 128], [1, C]]))

            # ---------------- persistent big tensors ----------------
            qT = persist.tile([128, KT, T], f32r)       # q^T  head-pair-major
            kTt = persist.tile([128, KT, T], f32r)      # k^T
            vp = persist.tile([128, TC, H * 65], bf16)  # v' + ones column
            yT = persist.tile([128, KT, T], f32r)       # y^T (unnormalized)

            nc.vector.memset(vp, 1.0)  # ones cols survive

            for _rep in range(reps):
                # ---------------- phase 1: xT + qkv ----------------
                p1pool = tc.tile_pool(name="p1", bufs=1)
                p1 = p1pool.__enter__()
                xT = p1.tile([128, KT, T], f32r)            # x^T  [C, T]
                wn = p1.tile([128, KT, 2, 1152], f32r)      # W_attn natural

                # ---------------- phase 1a: transpose x ----------------
                with (
                    tc.tile_pool(name="xnat", bufs=3) as xnat_p,
                    tc.tile_pool(name="tp", bufs=4, space="PSUM") as tp_p,
                ):
                    for i in range(TC):
                        xnat = xnat_p.tile([128, C], f32)
                        nc.sync.dma_start(out=xnat, in_=x_d[128 * i:128 * (i + 1), :])
                        for g in range(2):
                            tp = tp_p.tile([128, 384], f32)
                            for cc in range(3):
                                cb = 3 * g + cc
                                nc.tensor.transpose(
                                    tp[:, 128 * cc:128 * (cc + 1)],
                                    xnat[:, 128 * cb:128 * (cb + 1)], ident)
                            dst = AP(tensor=xT.tensor,
                                     offset=xT.offset + (3 * g) * T + 128 * i,
                                     ap=[xT.ap[0], [T, 3], [1, 128]])
                            src = AP(tensor=tp.tensor, offset=tp.offset,
                                     ap=[tp.ap[0], [128, 3], [1, 128]])
                            nc.vector.tensor_copy(dst, src)

                # W_attn loaded in its natural layout, kk-major
                for kk in range(KT):
                    for half in range(2):
                        nc.sync.dma_start(
                            out=wn[:, kk, half, :],
                            in_=wa_d[128 * kk:128 * (kk + 1),
                                     1152 * half:1152 * (half + 1)])

                # ---------------- phase 1b: q^T, k^T, v' ----------------
                with tc.tile_pool(name="qkp", bufs=1, space="PSUM") as qkp:
                    for h in range(H):
                        a, s = h // 2, h % 2
                        half, c0 = h // 6, 192 * (h % 6)
                        for j in range(2):
                            psN = qkp.tile([128, 512], f32, tag="qk",
                                           name="psN", bufs=4)
                            for kk in range(KT):
                                st, sp = kk == 0, kk == KT - 1
                                nc.tensor.matmul(psN, wn[:, kk, half, c0:c0 + 128],
                                                 xT[:, kk, 512 * j:512 * (j + 1)],
                                                 start=st, stop=sp)
                            nc.scalar.activation(
                                qT[64 * s:64 * (s + 1), a, 512 * j:512 * (j + 1)],
                                psN[0:64, :], Act.Identity,
                                bias=bqk[0:64, h:h + 1])
                            nc.vector.tensor_scalar_add(
                                kTt[64 * s:64 * (s + 1), a, 512 * j:512 * (j + 1)],
                                psN[64:128, :], bqk[64:128, h:h + 1])

                    # v token-major: lhsT = xT chunk, rhs = strided natural Wv
                    for i in range(TC):
                        for half in range(2):
                            pv = qkp.tile([128, 384], f32, tag="v",
                                          name="pv", bufs=4)
                            for kk in range(KT):
                                st, sp = kk == 0, kk == KT - 1
                                wt = wn[:, kk, half, :]
                                rhs = AP(tensor=wt.tensor, offset=wt.offset + 128,
                                         ap=[wt.ap[0], [192, 6], [1, 64]])
                                nc.tensor.matmul(pv,
                                                 xT[:, kk, 128 * i:128 * (i + 1)],
                                                 rhs, start=st, stop=sp)
                            dst = AP(tensor=vp.tensor,
                                     offset=vp.offset + i * (H * 65) + half * 6 * 65,
                                     ap=[vp.ap[0], [65, 6], [1, 64]])
                            src = AP(tensor=pv.tensor, offset=pv.offset,
                                     ap=[pv.ap[0], [64, 6], [1, 64]])
                            bsrc = AP(tensor=bv_b.tensor,
                                      offset=bv_b.offset + half * 384,
                                      ap=[bv_b.ap[0], [64, 6], [1, 64]])
                            nc.vector.tensor_add(dst, src, bsrc)

                p1pool.__exit__(None, None, None)

                # ---------------- phase 2: attention ----------------
                p2pool = tc.tile_pool(name="p2", bufs=1)
                p2 = p2pool.__enter__()
                wp = p2.tile([128, KT, C], f32r)
                for kk in range(KT):
                    nc.sync.dma_start(out=wp[:, kk, :],
                                      in_=wp_d[128 * kk:128 * (kk + 1), :])
                # attention probability buffers; [v][s]: v = pair parity
                # (double-buffered across pairs), s = head parity slot.
                es0 = [[p2.tile([128, 4 * 512], bf16, name=f"es0_{v}_{s}")
                        for s in range(2)] for v in range(2)]
                es1 = [[p2.tile([128, 8 * 512], bf16, name=f"es1_{v}_{s}")
                        for s in range(2)] for v in range(2)]
                with (
                    tc.tile_pool(name="sp", bufs=3, space="PSUM") as sp_p,
                    tc.tile_pool(name="yp", bufs=2, space="PSUM") as yp_p,
                    tc.tile_pool(name="bct", bufs=3) as bct_p,
                    tc.tile_pool(name="rec", bufs=4) as rec_p,
                ):
                    for a in range(NPAIR):
                        pv_ = a % 2
                        for j in range(2):
                            nm = 4 * (j + 1)          # causal k-chunks for q tile j
                            es = es0[pv_] if j == 0 else es1[pv_]
                            for w0 in range(0, nm, 2):   # waves of 2 k-chunks
                                spt = [sp_p.tile([128, 1024], f32, tag="s",
                                                 name=f"spt{s}") for s in range(2)]
                                for s in range(2):
                                    r0, r1 = 64 * s, 64 * (s + 1)
                                    for mi, m in enumerate((w0, w0 + 1)):
                                        nc.tensor.matmul(
                                            spt[s][:, 512 * mi:512 * (mi + 1)],
                                            kTt[r0:r1, a, 128 * m:128 * (m + 1)],
                                            qT[r0:r1, a, 512 * j:512 * (j + 1)],
                                            start=True, stop=True,
                                            tile_position=(64 * s, 0))
                                for s in range(2):
                                    nc.scalar.activation(
                                        es[s][:, 512 * w0:512 * (w0 + 2)], spt[s],
                                        Act.Exp, scale=0.125)
                                # causal fixups for this wave
                                for s in range(2):
                                    est = es[s]
                                    for m in (w0, w0 + 1):
                                        qs = 128 * m - 512 * j
                                        if qs > 0:
                                            nc.gpsimd.memset(
                                                est[:, 512 * m:512 * m + qs], 0.0)
                                        if 0 <= 128 * m - 512 * j < 512:
                                            dc = 640 * m - 512 * j
                                            dv = est[:, dc:dc + 128]
                                            nc.vector.tensor_mul(dv, dv, tri_bf)

                            # y'T = v'^T att^T accumulated over k-chunks
                            for s in range(2):
                                hh = 2 * a + s
                                est = es[s]
                                ypsum = yp_p.tile([128, 512], f32, tag="y",
                                                  name=f"ypsum{s}")
                                for m in range(nm):
                                    qs = min(max(128 * m - 512 * j, 0), 256)
                                    nc.tensor.matmul(
                                        ypsum[0:65, qs:512],
                                        vp[:, m, 65 * hh:65 * hh + 65],
                                        est[:, 512 * m + qs:512 * (m + 1)],
                                        start=(m == 0), stop=(m == nm - 1))
                                # unnormalized y' straight to yT; denominator
                                # row to the partition-packed den tile
                                dsb = rec_p.tile([1, 512], f32, name="dsb")
                                nc.vector.tensor_copy(dsb, ypsum[64:65, :])
                                rec = rec_p.tile([1, 512], f32, name="rec")
                                nc.vector.reciprocal_approx_fast(rec, dsb)
                                bc = bct_p.tile([128, 512], f32, name="bc")
                                nc.gpsimd.partition_broadcast(bc, rec)
                                nc.vector.tensor_mul(
                                    yT[64 * s:64 * (s + 1), a,
                                       512 * j:512 * (j + 1)],
                                    ypsum[0:64, :], bc[0:64, :])

                # ---------------- phase 3: output projection ----------------
                with (
                    tc.tile_pool(name="pp", bufs=3, space="PSUM") as pp_p,
                    tc.tile_pool(name="osb", bufs=3) as osb_p,
                ):
                    for i in range(TC):
                        po = pp_p.tile([128, C], f32)
                        for kk in range(KT):
                            st, sp = kk == 0, kk == KT - 1
                            lhsT = yT[:, kk, 128 * i:128 * (i + 1)]
                            nc.tensor.matmul(po[:, 0:512], lhsT, wp[:, kk, 0:512],
                                             start=st, stop=sp)
                            nc.tensor.matmul(po[:, 512:768], lhsT, wp[:, kk, 512:768],
                                             start=st, stop=sp)
                        osb = osb_p.tile([128, C], f32)
                        nc.vector.tensor_add(osb, po, bp_b)
                        nc.sync.dma_start(out=y_d[128 * i:128 * (i + 1), :], in_=osb)
                p2pool.__exit__(None, None, None)

    nc.compile()
    return nc


_NC_CACHE = {}


def _get_nc():
    if "nc" not in _NC_CACHE:
        _NC_CACHE["nc"] = build_nc()
    return _NC_CACHE["nc"]


def kernel(x, W_attn, b_attn, W_proj, b_proj):
    from concourse.bass_utils import run_bass_kernel_spmd

    nc = _get_nc()
    x = np.asarray(x, dtype=np.float32)
    tri = np.triu(np.ones((128, 128), dtype=np.float32))
    shared = {
        "w_attn": np.ascontiguousarray(np.asarray(W_attn, dtype=np.float32)),
        "w_proj": np.ascontiguousarray(np.asarray(W_proj, dtype=np.float32)),
        "b_attn": np.ascontiguousarray(np.asarray(b_attn, dtype=np.float32)),
        "b_proj": np.ascontiguousarray(np.asarray(b_proj, dtype=np.float32)),
        "tri": tri,
    }
    in_maps = [dict(shared, x=np.ascontiguousarray(x[c])) for c in range(B)]
    res = run_bass_kernel_spmd(nc, in_maps, list(range(B)))
    out = np.stack([res.results[c]["y"] for c in range(B)], axis=0)
    return out.astype(np.float32)
